# revision 11
# baseline (speedup 1.0000x reference)
"""ArcMarginProduct + cross-entropy loss, vocab-parallel over 8 NeuronCores.

Math: the reference computes
    cos[b,v] = <x_b/|x_b|, w_v/|w_v|>,  clip to [-1+eps, 1-eps]
    logits   = cos(arccos(cos) + M*onehot(labels))
    loss     = mean(logsumexp(logits, axis=1) - logits[b, label_b])
For v != label_b, cos(arccos(c)) == c, so the only place arccos/cos matter is
the single label column per row -- handled exactly on the host (O(B*D) work).
The device computes, per vocabulary shard, S_partial[b] = sum_v exp(cos[b,v])
(raw, no margin). |cos|<=1 always, so no max-shift is needed for stability.
Host then corrects the label term: S_adj = S - exp(c_label) + exp(c_adj),
loss = mean(log(S_adj) - c_adj).

Sharding: weight columns split V=100000 -> 8 x 12500, padded with zero
columns to 12544 per core (pad contributions are constant and subtracted
exactly on the host).

Device kernel (per core): both operands are L2-normalized ON THE HOST and
shipped as fp8, so the PSUM matmul result is exactly SX*SW*cos and the exp
scale is one scalar constant. Layout: batch rows on PSUM partitions (8
b-tiles of 128), classes on the free axis, so the per-row sum over classes
is a free-axis reduction the ScalarE activation produces for free via
accum_out. Per [128, 2048] class group (4 PSUM banks, double-buffered):
DoubleRow fp8 matmuls (256-deep contraction, x stationary) accumulate over
D; then the group is consumed column-split by TWO engines in parallel --
ScalarE runs Exp+accum_out on the first ACT_COLS columns while VectorE runs
a Schraudolph bit-trick exp (bits(bf16(e^z)) ~= round(A*P + B), ~1.3% rms,
~0.1% mean error) on the rest, with one batched free-axis reduce per b-tile.
No SBUF-side accumulation pass exists at all; the host sums 7 columns per
b-tile. PE warm-up matmuls during the weight-DMA lead-in keep the HAM clock
gate at 8/8 when the real matmuls start.
"""

import math
import sys

if "/opt/trn_rl_repo" not in sys.path:
    sys.path.insert(0, "/opt/trn_rl_repo")

import numpy as np
import ml_dtypes

import concourse.bass as bass
import concourse.mybir as mybir
import concourse.tile as tile
from concourse.bass_utils import run_bass_kernel_spmd

B, D, V = 1024, 512, 100000
NCORES = 8
VS = V // NCORES           # 12500 true classes per core
VSP = 12544                # padded classes per core
KB = D // 128              # 4 contraction blocks
NBT = B // 128             # 8 batch tiles (PSUM partition dim)
GV = 2048                  # classes per big PSUM group (4 banks)
NVG = 7                    # groups per batch tile: 6 big + 1 small (256)
SMALL = VSP - 6 * GV       # 256
MARGIN = 0.4
EPS = 1e-7
SX = 32.0                  # fp8 scale for x_norm
SW = 256.0                 # fp8 scale for w_norm
EXP_SCALE = 1.0 / (SX * SW)

# column split inside each big group: ScalarE takes [0, ACT_COLS), VectorE
# the rest; small groups split at SMALL_ACT
ACT_COLS = 1264
DVE_COLS = GV - ACT_COLS   # 784
SMALL_ACT = 128
SMALL_DVE = SMALL - SMALL_ACT

# Schraudolph constants: bits(bf16(exp(P*EXP_SCALE))) ~= round(SCH_A*P+SCH_B)
SCH_A = 128.0 * math.log2(math.e) * EXP_SCALE
SCH_B = 127.0 * 128.0 - 2.0
# exact value a zero-padded class contributes through the Schraudolph path
PAD_VAL = float(
    np.array([int(round(SCH_B))], dtype=np.uint16)
    .view(ml_dtypes.bfloat16)[0]
    .astype(np.float64)
)

# graded weight-DMA chunks (class-column bounds): small first chunks so the
# first matmuls start early, big later ones to keep the DGE count low
DMA_BOUNDS = [0, 512, 1024, 2048, 3584, 5632, 8192, 12544]
WARMUP_MM = 16             # dummy matmuls to warm the PE HAM clock gate

BF16 = mybir.dt.bfloat16
FP8 = mybir.dt.float8e4
U16 = mybir.dt.uint16
F32 = mybir.dt.float32
AF = mybir.ActivationFunctionType
DR = mybir.MatmulPerfMode.DoubleRow
ALU = mybir.AluOpType
AX = mybir.AxisListType

_nc_cache = {}


def _split_multi_waits(nc):
    """This toolchain's walrus accepts at most ONE semaphore wait per
    instruction, but TileContext attaches one wait per producing processor.
    Rewrite any instruction carrying N>1 waits into N-1 same-engine NoOps
    (one wait each) inserted immediately before it; same-engine program order
    keeps the semantics identical."""
    uid = 0
    for f in nc.m.functions:
        for bb in f.blocks:
            insts = bb.instructions
            i = 0
            while i < len(insts):
                inst = insts[i]
                si = inst.sync_info
                if si is not None and len(si.on_wait) > 1:
                    waits = list(si.on_wait)
                    for w in waits[:-1]:
                        uid += 1
                        nop = mybir.InstNoOp(
                            name=f"{inst.name}-wsplit{uid}",
                            engine=inst.engine,
                            sync_info=mybir.SyncInfo(on_wait=[w], on_update=[]),
                            bass_nofuse=True,
                        )
                        insts.insert(i, nop)
                        i += 1
                    inst.sync_info = mybir.SyncInfo(
                        on_wait=[waits[-1]], on_update=list(si.on_update)
                    )
                i += 1


def _build_nc():
    nc = bass.Bass(target_bir_lowering=False)
    xT = nc.declare_dram_parameter("xT", [D, B], FP8, isOutput=False)
    w = nc.declare_dram_parameter("w", [D, VSP], FP8, isOutput=False)
    # per-(partition, b-tile, group) partial sums: ScalarE's 56 columns,
    # then VectorE's 56
    acc_out = nc.declare_dram_parameter("acc", [128, 2 * NBT * NVG], F32, isOutput=True)

    xT_r = xT.rearrange("(k p) b -> p k b", p=128)
    w_r = w.rearrange("(k p) v -> p k v", p=128)

    with tile.TileContext(nc) as tc:
        with (
            tc.tile_pool(name="persist", bufs=1) as persist,
            tc.tile_pool(name="pm", bufs=2, space="PSUM") as pm_pool,
        ):
            xt = persist.tile([128, KB, B], FP8, tag="xt")
            nc.sync.dma_start(xt[:, :, :], xT_r[:, :, :])
            warm = persist.tile([128, 128], FP8, tag="warm")
            nc.vector.memset(warm[:, :], 0.0625)
            # whole weight shard stays resident in SBUF (fp8: ~6.3 MB)
            wall = persist.tile([128, KB, VSP], FP8, tag="wall")
            for c in range(len(DMA_BOUNDS) - 1):
                v0, v1 = DMA_BOUNDS[c], DMA_BOUNDS[c + 1]
                nc.sync.dma_start(wall[:, :, v0:v1], w_r[:, :, v0:v1])
            scr_a = persist.tile([128, ACT_COLS], BF16, tag="scr_a")
            scr_d = persist.tile([128, NVG, DVE_COLS], BF16, tag="scr_d")
            sums_a = persist.tile([128, NBT * NVG], F32, tag="sums_a")
            sums_d = persist.tile([128, NBT * NVG], F32, tag="sums_d")

            for bt in range(NBT):
                bs = slice(bt * 128, (bt + 1) * 128)
                for vg in range(NVG):
                    nv = GV if vg < 6 else SMALL
                    c0 = vg * GV
                    col = bt * NVG + vg
                    psum = pm_pool.tile([128, GV], F32, tag="pm")
                    if bt == 0 and vg == 0:
                        # keep the PE busy during the weight-DMA lead-in so
                        # the HAM clock gate is 8/8 when real matmuls start
                        for _ in range(WARMUP_MM):
                            nc.tensor.matmul(psum[:, :128], warm[:, :], warm[:, :])
                    for g in range(KB // 2):
                        for c in range(0, nv, 512):
                            cw = min(512, nv - c)
                            nc.tensor.matmul(
                                psum[:, c : c + cw],
                                xt[:, 2 * g : 2 * g + 2, bs],
                                wall[:, 2 * g : 2 * g + 2, c0 + c : c0 + c + cw],
                                start=(g == 0),
                                stop=(g == KB // 2 - 1),
                                perf_mode=DR,
                            )
                    na = ACT_COLS if vg < 6 else SMALL_ACT
                    nd = nv - na
                    nc.scalar.activation(
                        scr_a[:, :na],
                        psum[:, :na],
                        AF.Exp,
                        scale=EXP_SCALE,
                        accum_out=sums_a[:, col : col + 1],
                    )
                    nc.vector.tensor_scalar(
                        scr_d.bitcast(U16)[:, vg, :nd],
                        psum[:, na:nv],
                        SCH_A,
                        SCH_B,
                        op0=ALU.mult,
                        op1=ALU.add,
                    )
                    if vg == 5:
                        # one batched reduce for this b-tile's six big groups
                        nc.vector.tensor_reduce(
                            sums_d[:, bt * NVG : bt * NVG + 6],
                            scr_d[:, :6, :],
                            axis=AX.X,
                            op=ALU.add,
                        )
                    elif vg == 6:
                        nc.vector.tensor_reduce(
                            sums_d[:, col : col + 1],
                            scr_d[:, 6, :SMALL_DVE],
                            axis=AX.X,
                            op=ALU.add,
                        )

            nc.sync.dma_start(acc_out[:, : NBT * NVG], sums_a[:, :])
            nc.sync.dma_start(acc_out[:, NBT * NVG :], sums_d[:, :])

    _split_multi_waits(nc)
    return nc


def _get_nc():
    if "nc" not in _nc_cache:
        _nc_cache["nc"] = _build_nc()
    return _nc_cache["nc"]


def run_device(in_maps, **kwargs):
    return run_bass_kernel_spmd(_get_nc(), in_maps, list(range(NCORES)), **kwargs)


def make_in_maps(input, weight):
    x = np.asarray(input, dtype=np.float32)
    w = np.asarray(weight, dtype=np.float32)
    x_norm = x / np.maximum(np.linalg.norm(x, axis=1, keepdims=True), 1e-12)
    w_norm = w / np.maximum(np.linalg.norm(w, axis=0, keepdims=True), 1e-12)
    np_dt = ml_dtypes.float8_e4m3
    xT8 = np.ascontiguousarray(x_norm.T * np.float32(SX)).astype(np_dt)
    w8 = (w_norm * np.float32(SW)).astype(np_dt)
    pad = np.zeros((D, VSP - VS), dtype=np_dt)
    return [
        {
            "xT": xT8,
            "w": np.ascontiguousarray(
                np.concatenate([w8[:, i * VS : (i + 1) * VS], pad], axis=1)
            ),
        }
        for i in range(NCORES)
    ]


def finalize(results, input, weight, labels):
    """Host epilogue: reduce the per-(b-tile, group) partial sums, remove the
    zero-pad columns' exact contributions, and apply the exact label-margin
    correction (O(B*D) work)."""
    x = np.asarray(input, dtype=np.float64)
    w = np.asarray(weight, dtype=np.float32)
    lab = np.asarray(labels).astype(np.int64)

    S = np.zeros(B, dtype=np.float64)
    for i in range(NCORES):
        acc = results[i]["acc"].astype(np.float64)  # [128, 2*56]
        both = acc[:, : NBT * NVG] + acc[:, NBT * NVG :]
        part = both.reshape(128, NBT, NVG).sum(axis=2)  # [128, NBT]
        S += part.T.reshape(B)
    # zero-pad classes sit in the VectorE share of each small group and each
    # contributed exactly PAD_VAL per batch row
    S -= NCORES * (VSP - VS) * PAD_VAL

    x_norm = x / np.maximum(np.linalg.norm(x, axis=1, keepdims=True), 1e-12)
    wl = w[:, lab].astype(np.float64)                    # [D, B]
    wln = np.maximum(np.sqrt((wl * wl).sum(axis=0)), 1e-12)
    c = (x_norm.T * wl).sum(axis=0) / wln                # label cosines
    c = np.clip(c, -1.0 + EPS, 1.0 - EPS)
    c_adj = np.cos(np.arccos(c) + MARGIN)
    S_adj = S - np.exp(c) + np.exp(c_adj)
    logz = np.log(S_adj)
    loss = np.mean(logz - c_adj)
    return np.asarray(loss, dtype=np.float32)


def kernel(input, weight, labels):
    in_maps = make_in_maps(input, weight)
    res = run_device(in_maps)
    return finalize(res.results, input, weight, labels)


# revision 16
# speedup vs baseline: 1.0420x; 1.0420x over previous
"""ArcMarginProduct + cross-entropy loss, vocab-parallel over 8 NeuronCores.

Math: the reference computes
    cos[b,v] = <x_b/|x_b|, w_v/|w_v|>,  clip to [-1+eps, 1-eps]
    logits   = cos(arccos(cos) + M*onehot(labels))
    loss     = mean(logsumexp(logits, axis=1) - logits[b, label_b])
For v != label_b, cos(arccos(c)) == c, so the only place arccos/cos matter is
the single label column per row -- handled exactly on the host (O(B*D) work).
The device computes, per vocabulary shard, S_partial[b] = sum_v exp(cos[b,v])
(raw, no margin). |cos|<=1 always, so no max-shift is needed for stability.
Host then corrects the label term: S_adj = S - exp(c_label) + exp(c_adj),
loss = mean(log(S_adj) - c_adj).

Sharding: weight columns split V=100000 -> 8 x 12500, padded with zero
columns to 12544 per core (pad contributions are constant and subtracted
exactly on the host).

Device kernel (per core): both operands are L2-normalized ON THE HOST and
shipped as fp8, so the PSUM matmul result is exactly SX*SW*cos and the exp
scale is one scalar constant. Layout: batch rows on PSUM partitions (8
b-tiles of 128), classes on the free axis, so the per-row sum over classes
is a free-axis reduction the ScalarE activation produces for free via
accum_out. Per [128, 2048] class group (4 PSUM banks, double-buffered):
DoubleRow fp8 matmuls (256-deep contraction, x stationary) accumulate over
D; then the group is consumed column-split by TWO engines in parallel --
ScalarE runs Exp+accum_out on the first ACT_COLS columns while VectorE runs
a Schraudolph bit-trick exp (bits(bf16(e^z)) ~= round(A*P + B), ~1.3% rms,
~0.1% mean error) on the rest, with one batched free-axis reduce per b-tile.
No SBUF-side accumulation pass exists at all; the host sums 7 columns per
b-tile. PE warm-up matmuls during the weight-DMA lead-in keep the HAM clock
gate at 8/8 when the real matmuls start.
"""

import math
import sys

if "/opt/trn_rl_repo" not in sys.path:
    sys.path.insert(0, "/opt/trn_rl_repo")

import numpy as np
import ml_dtypes

import concourse.bass as bass
import concourse.mybir as mybir
import concourse.tile as tile
from concourse.bass_utils import run_bass_kernel_spmd

B, D, V = 1024, 512, 100000
NCORES = 8
VS = V // NCORES           # 12500 true classes per core
VSP = 12544                # padded classes per core
KB = D // 128              # 4 contraction blocks
NBT = B // 128             # 8 batch tiles (PSUM partition dim)
GV = 2048                  # classes per big PSUM group (4 banks)
NVG = 7                    # groups per batch tile: 6 big + 1 small (256)
SMALL = VSP - 6 * GV       # 256
MARGIN = 0.4
EPS = 1e-7
SX = 32.0                  # fp8 scale for x_norm
SW = 256.0                 # fp8 scale for w_norm
EXP_SCALE = 1.0 / (SX * SW)

# column split inside each big group: ScalarE takes [0, ACT_COLS) with
# exp+accum_out, VectorE the rest via the Schraudolph bit-trick; the small
# (256-col) groups run entirely on ScalarE
ACT_COLS = 1216
DVE_COLS = GV - ACT_COLS   # 832

# Schraudolph constants: bits(bf16(exp(P*EXP_SCALE))) ~= round(SCH_A*P+SCH_B)
SCH_A = 128.0 * math.log2(math.e) * EXP_SCALE
SCH_B = 127.0 * 128.0 - 2.0
# zero-pad classes live in the ScalarE small-group share: exp(0) = 1 exactly
PAD_VAL = 1.0

# graded weight-DMA chunks (class-column bounds): small first chunks so the
# first matmuls start early, big later ones to keep the DGE count low
DMA_BOUNDS = [0, 512, 1024, 2048, 3584, 5632, 8192, 12544]
WARMUP_MM = 18             # dummy matmuls to warm the PE HAM clock gate

BF16 = mybir.dt.bfloat16
FP8 = mybir.dt.float8e4
U16 = mybir.dt.uint16
F32 = mybir.dt.float32
AF = mybir.ActivationFunctionType
DR = mybir.MatmulPerfMode.DoubleRow
ALU = mybir.AluOpType
AX = mybir.AxisListType

_nc_cache = {}


def _split_multi_waits(nc):
    """This toolchain's walrus accepts at most ONE semaphore wait per
    instruction, but TileContext attaches one wait per producing processor.
    Rewrite any instruction carrying N>1 waits into N-1 same-engine NoOps
    (one wait each) inserted immediately before it; same-engine program order
    keeps the semantics identical."""
    uid = 0
    for f in nc.m.functions:
        for bb in f.blocks:
            insts = bb.instructions
            i = 0
            while i < len(insts):
                inst = insts[i]
                si = inst.sync_info
                if si is not None and len(si.on_wait) > 1:
                    waits = list(si.on_wait)
                    for w in waits[:-1]:
                        uid += 1
                        nop = mybir.InstNoOp(
                            name=f"{inst.name}-wsplit{uid}",
                            engine=inst.engine,
                            sync_info=mybir.SyncInfo(on_wait=[w], on_update=[]),
                            bass_nofuse=True,
                        )
                        insts.insert(i, nop)
                        i += 1
                    inst.sync_info = mybir.SyncInfo(
                        on_wait=[waits[-1]], on_update=list(si.on_update)
                    )
                i += 1


def _build_nc():
    nc = bass.Bass(target_bir_lowering=False)
    xT = nc.declare_dram_parameter("xT", [D, B], FP8, isOutput=False)
    w = nc.declare_dram_parameter("w", [D, VSP], FP8, isOutput=False)
    # per-(partition, b-tile, group) partial sums: ScalarE's 7 columns per
    # b-tile (6 big + small), then VectorE's 6 per b-tile
    acc_out = nc.declare_dram_parameter(
        "acc", [128, NBT * NVG + NBT * 6], F32, isOutput=True
    )

    xT_r = xT.rearrange("(k p) b -> p k b", p=128)
    w_r = w.rearrange("(k p) v -> p k v", p=128)

    with tile.TileContext(nc) as tc:
        with (
            tc.tile_pool(name="persist", bufs=1) as persist,
            tc.tile_pool(name="pm", bufs=2, space="PSUM") as pm_pool,
        ):
            xt = persist.tile([128, KB, B], FP8, tag="xt")
            nc.sync.dma_start(xt[:, :, :], xT_r[:, :, :])
            warm = persist.tile([128, 128], FP8, tag="warm")
            nc.vector.memset(warm[:, :], 0.0625)
            # whole weight shard stays resident in SBUF (fp8: ~6.3 MB)
            wall = persist.tile([128, KB, VSP], FP8, tag="wall")
            for c in range(len(DMA_BOUNDS) - 1):
                v0, v1 = DMA_BOUNDS[c], DMA_BOUNDS[c + 1]
                nc.sync.dma_start(wall[:, :, v0:v1], w_r[:, :, v0:v1])
            scr_a = persist.tile([128, ACT_COLS], BF16, tag="scr_a")
            # double-buffered by b-tile parity so the batched reduces of
            # b-tile N can run while b-tile N+1's groups are being written
            scr_d = persist.tile([128, 2, 6, DVE_COLS], BF16, tag="scr_d")
            sums_a = persist.tile([128, NBT * NVG], F32, tag="sums_a")
            sums_d = persist.tile([128, NBT * 6], F32, tag="sums_d")

            def reduce_half(bt, half):
                s0 = 3 * half
                nc.vector.tensor_reduce(
                    sums_d[:, bt * 6 + s0 : bt * 6 + s0 + 3],
                    scr_d[:, bt % 2, s0 : s0 + 3, :],
                    axis=AX.X,
                    op=ALU.add,
                )

            for bt in range(NBT):
                bs = slice(bt * 128, (bt + 1) * 128)
                for vg in range(NVG):
                    nv = GV if vg < 6 else SMALL
                    c0 = vg * GV
                    psum = pm_pool.tile([128, GV], F32, tag="pm")
                    if bt == 0 and vg == 0:
                        # keep the PE busy during the weight-DMA lead-in so
                        # the HAM clock gate is 8/8 when real matmuls start
                        for _ in range(WARMUP_MM):
                            nc.tensor.matmul(psum[:, :128], warm[:, :], warm[:, :])
                    for g in range(KB // 2):
                        for c in range(0, nv, 512):
                            cw = min(512, nv - c)
                            nc.tensor.matmul(
                                psum[:, c : c + cw],
                                xt[:, 2 * g : 2 * g + 2, bs],
                                wall[:, 2 * g : 2 * g + 2, c0 + c : c0 + c + cw],
                                start=(g == 0),
                                stop=(g == KB // 2 - 1),
                                perf_mode=DR,
                            )
                    if vg == 6:
                        # small group: ScalarE only
                        nc.scalar.activation(
                            scr_a[:, :SMALL],
                            psum[:, :SMALL],
                            AF.Exp,
                            scale=EXP_SCALE,
                            accum_out=sums_a[:, bt * NVG + 6 : bt * NVG + 7],
                        )
                        continue
                    nc.scalar.activation(
                        scr_a[:, :],
                        psum[:, :ACT_COLS],
                        AF.Exp,
                        scale=EXP_SCALE,
                        accum_out=sums_a[:, bt * NVG + vg : bt * NVG + vg + 1],
                    )
                    nc.vector.tensor_scalar(
                        scr_d[:, bt % 2, vg, :].bitcast(U16),
                        psum[:, ACT_COLS:],
                        SCH_A,
                        SCH_B,
                        op0=ALU.mult,
                        op1=ALU.add,
                    )
                    if bt == NBT - 1:
                        # last b-tile: reduce each group right away (short
                        # ops, keeps the tail chain minimal)
                        nc.vector.tensor_reduce(
                            sums_d[:, bt * 6 + vg : bt * 6 + vg + 1],
                            scr_d[:, bt % 2, vg, :],
                            axis=AX.X,
                            op=ALU.add,
                        )
                    if bt > 0 and vg < 2:
                        # previous b-tile's batched reduces, deferred two
                        # groups so they never sit ahead of a PSUM release
                        reduce_half(bt - 1, vg)

            nc.sync.dma_start(acc_out[:, : NBT * NVG], sums_a[:, :])
            nc.sync.dma_start(acc_out[:, NBT * NVG :], sums_d[:, :])

    _split_multi_waits(nc)
    return nc


def _get_nc():
    if "nc" not in _nc_cache:
        _nc_cache["nc"] = _build_nc()
    return _nc_cache["nc"]


def run_device(in_maps, **kwargs):
    return run_bass_kernel_spmd(_get_nc(), in_maps, list(range(NCORES)), **kwargs)


def make_in_maps(input, weight):
    x = np.asarray(input, dtype=np.float32)
    w = np.asarray(weight, dtype=np.float32)
    x_norm = x / np.maximum(np.linalg.norm(x, axis=1, keepdims=True), 1e-12)
    w_norm = w / np.maximum(np.linalg.norm(w, axis=0, keepdims=True), 1e-12)
    np_dt = ml_dtypes.float8_e4m3
    xT8 = np.ascontiguousarray(x_norm.T * np.float32(SX)).astype(np_dt)
    w8 = (w_norm * np.float32(SW)).astype(np_dt)
    pad = np.zeros((D, VSP - VS), dtype=np_dt)
    return [
        {
            "xT": xT8,
            "w": np.ascontiguousarray(
                np.concatenate([w8[:, i * VS : (i + 1) * VS], pad], axis=1)
            ),
        }
        for i in range(NCORES)
    ]


def finalize(results, input, weight, labels):
    """Host epilogue: reduce the per-(b-tile, group) partial sums, remove the
    zero-pad columns' exact contributions, and apply the exact label-margin
    correction (O(B*D) work)."""
    x = np.asarray(input, dtype=np.float64)
    w = np.asarray(weight, dtype=np.float32)
    lab = np.asarray(labels).astype(np.int64)

    S = np.zeros(B, dtype=np.float64)
    for i in range(NCORES):
        acc = results[i]["acc"].astype(np.float64)  # [128, 56 + 48]
        part = acc[:, : NBT * NVG].reshape(128, NBT, NVG).sum(axis=2)
        part += acc[:, NBT * NVG :].reshape(128, NBT, 6).sum(axis=2)
        S += part.T.reshape(B)
    # zero-pad classes sit in the ScalarE small-group share: exp(0) = 1
    S -= NCORES * (VSP - VS) * PAD_VAL

    x_norm = x / np.maximum(np.linalg.norm(x, axis=1, keepdims=True), 1e-12)
    wl = w[:, lab].astype(np.float64)                    # [D, B]
    wln = np.maximum(np.sqrt((wl * wl).sum(axis=0)), 1e-12)
    c = (x_norm.T * wl).sum(axis=0) / wln                # label cosines
    c = np.clip(c, -1.0 + EPS, 1.0 - EPS)
    c_adj = np.cos(np.arccos(c) + MARGIN)
    S_adj = S - np.exp(c) + np.exp(c_adj)
    logz = np.log(S_adj)
    loss = np.mean(logz - c_adj)
    return np.asarray(loss, dtype=np.float32)


def kernel(input, weight, labels):
    in_maps = make_in_maps(input, weight)
    res = run_device(in_maps)
    return finalize(res.results, input, weight, labels)
